# revision 20
# baseline (speedup 1.0000x reference)
"""Multi-head attention (projections + softmax attention + output proj) on 8
Trainium2 NeuronCores, data-parallel over the batch dim (16 batches -> 2 per
core).

Math (per batch item, H=12 heads, C=64):
    qp = q @ Wq.T + bq        (same k, v)
    S_h = (qp_h * 1/8) @ kp_h.T            [Lq, Lkv]
    P_h = softmax over kv
    mix_h = P_h @ vp_h
    out = concat_h(mix_h) @ Wo.T + bo
Outputs: (out, qh, kh, vh) where qh/kh/vh are the projected tensors reshaped
to [N, L, H, C].

Kernel layout strategy (per core):
  - activations are PE-transposed to xT [d, t]; projections produce qpT/kpT
    in [e, t] layout (heads = partition slices) and vp in row layout,
    augmented with a ones column (vaug) so the PV matmul also produces the
    softmax denominator for free.
  - S^T = K Q^T is computed directly in [kv, q] layout (no P transposes);
    exp runs on ScalarE with the 1/8 scale folded in; PV accumulates
    mix_aug^T = Vaug^T P^T in PSUM; normalization multiplies by the
    broadcast reciprocal denominator during PSUM evacuation.
  - output projection contracts mixT against WoT producing row-layout out.
"""

from contextlib import ExitStack

import numpy as np

import concourse.bass as bass
import concourse.tile as tile
from concourse import bacc, mybir
from concourse.bass_utils import run_bass_kernel_spmd
from concourse.masks import make_identity

FP = mybir.dt.float32
NCORES = 8
B = 2  # batch items per core
L = 1024  # sequence length (q and kv)
D = 768  # model dim
H = 12  # heads
C = 64  # head channels
DC = D // 128  # 6 chunks of the contraction dim
TT = L // 128  # 8 token tiles per batch item
SCALE = 1.0 / np.sqrt(C).astype(np.float32)  # 0.125

EXP = mybir.ActivationFunctionType.Exp


def _rows(t):
    # [B, L, D] dram tensor -> [B*TT, 128, D] token-tile view
    return t.ap().rearrange("b l d -> (b l) d").rearrange("(t p) d -> t p d", p=128)


def _emit(nc, tc, io, dbg=None):
    ctx = ExitStack()
    sync = nc.sync
    act = nc.scalar
    dve = nc.vector
    mm = nc.tensor.matmul

    q_rows, k_rows, v_rows = _rows(io["q"]), _rows(io["k"]), _rows(io["v"])
    out_rows, qh_rows, kh_rows, vh_rows = (
        _rows(io["out"]),
        _rows(io["qh"]),
        _rows(io["kh"]),
        _rows(io["vh"]),
    )
    w_dram = {n: io[n].ap() for n in ("Wq", "Wk", "Wv", "Wo")}

    singles = ctx.enter_context(tc.tile_pool(name="singles", bufs=1))
    wpool = ctx.enter_context(tc.tile_pool(name="wpool", bufs=2))
    big6 = ctx.enter_context(tc.tile_pool(name="big6", bufs=2))
    pkt = ctx.enter_context(tc.tile_pool(name="pkt", bufs=1))
    vpool = ctx.enter_context(tc.tile_pool(name="vpool", bufs=1))
    xrows = ctx.enter_context(tc.tile_pool(name="xrows", bufs=4))
    orows = ctx.enter_context(tc.tile_pool(name="orows", bufs=3))
    ptpool = ctx.enter_context(tc.tile_pool(name="ptpool", bufs=3))
    rpool = ctx.enter_context(tc.tile_pool(name="rpool", bufs=2))
    mpool = ctx.enter_context(tc.tile_pool(name="mpool", bufs=2))
    ps_qk = ctx.enter_context(tc.tile_pool(name="ps_qk", bufs=2, space="PSUM"))
    ps_pv = ctx.enter_context(tc.tile_pool(name="ps_pv", bufs=2, space="PSUM"))
    ps_ms = ctx.enter_context(tc.tile_pool(name="ps_ms", bufs=2, space="PSUM"))

    # --- constants ---------------------------------------------------------
    identity = singles.tile([128, 128], FP)
    make_identity(nc, identity)
    ones = singles.tile([1, 128], FP)
    dve.memset(ones, 1.0)
    bq_pt = singles.tile([128, DC], FP)  # bias per-partition layout [e_in, ec]
    nc.gpsimd.dma_start(out=bq_pt, in_=io["bq"].ap().rearrange("(j p) -> p j", p=128))
    bk_pt = singles.tile([128, DC], FP)
    nc.gpsimd.dma_start(out=bk_pt, in_=io["bk"].ap().rearrange("(j p) -> p j", p=128))

    def bias_bcast(name):
        row = xrows.tile([128, D], FP, name="xrow", tag="xrow")
        sync.dma_start(out=row[0:1, :], in_=io[name].ap().rearrange("(a d) -> a d", a=1))
        bc = singles.tile([128, D], FP, name=f"{name}_bc")
        for off, w in ((0, 512), (512, 256)):
            ps = ps_ms.tile([128, 512], FP, name="ms", tag="ms")
            mm(ps[:, :w], ones[0:1, 0:128], row[0:1, off : off + w], start=True, stop=True)  # noqa: E501
            dve.tensor_copy(bc[:, off : off + w], ps[:, :w])
        return bc

    bvb = bias_bcast("bv")
    bob = bias_bcast("bo")

    def load_weight_T(wname):
        """DMA W [e, d] rows, PE-transpose to wT [d_in(128), dc, e]."""
        wT = wpool.tile([128, DC, D], FP, name=f"{wname}T", tag="W")
        for g in range(2):  # groups of 3 e-chunks
            wr = []
            for j in range(3):
                t = xrows.tile([128, D], FP, name="wrow", tag="xrow")
                sync.dma_start(
                    out=t,
                    in_=w_dram[wname].rearrange("(t p) d -> t p d", p=128)[3 * g + j],
                )
                wr.append(t)
            for dc in range(DC):
                ps = ps_ms.tile([128, 512], FP, name="ms", tag="ms")
                for j in range(3):
                    nc.tensor.transpose(
                        ps[:, j * 128 : (j + 1) * 128],
                        wr[j][:, dc * 128 : (dc + 1) * 128],
                        identity,
                    )
                act.copy(wT[:, dc, g * 384 : (g + 1) * 384], ps[:, 0:384])
        return wT

    def load_xT(x_rows_view, b):
        """DMA activation rows, PE-transpose to xT [d_in(128), dc, t]."""
        xT = big6.tile([128, DC, L], FP, name="xT", tag="big")
        for g in range(2):  # groups of 4 token tiles
            xr = []
            for j in range(4):
                t = xrows.tile([128, D], FP, name="xrow", tag="xrow")
                sync.dma_start(out=t, in_=x_rows_view[b * TT + 4 * g + j])
                xr.append(t)
            for dc in range(DC):
                ps = ps_ms.tile([128, 512], FP, name="ms", tag="ms")
                for j in range(4):
                    nc.tensor.transpose(
                        ps[:, j * 128 : (j + 1) * 128],
                        xr[j][:, dc * 128 : (dc + 1) * 128],
                        identity,
                    )
                act.copy(xT[:, dc, g * 512 : (g + 1) * 512], ps)
        return xT

    for b in range(B):
        # ---- Q projection -> qpT [e, t], plus qh rows output -------------
        wT = load_weight_T("Wq")
        xT = load_xT(q_rows, b)
        qpT = pkt.tile([128, DC, L], FP, name="qpT", tag="qpT")
        for ec in range(DC):
            for th in range(2):
                ps = ps_ms.tile([128, 512], FP, name="ms", tag="ms")
                for dc in range(DC):
                    mm(
                        ps,
                        wT[:, dc, ec * 128 : (ec + 1) * 128],
                        xT[:, dc, th * 512 : (th + 1) * 512],
                        start=(dc == 0),
                        stop=(dc == DC - 1),
                    )
                dve.tensor_scalar_add(
                    qpT[:, ec, th * 512 : (th + 1) * 512], ps, bq_pt[:, ec : ec + 1]
                )

        def store_rows_from_T(srcT, dst_rows, b):
            for tt in range(TT):
                orow = orows.tile([128, D], FP, name="orow", tag="orow")
                for g in range(2):
                    ps = ps_ms.tile([128, 512], FP, name="ms", tag="ms")
                    for j in range(3):
                        nc.tensor.transpose(
                            ps[:, j * 128 : (j + 1) * 128],
                            srcT[:, 3 * g + j, tt * 128 : (tt + 1) * 128],
                            identity,
                        )
                    act.copy(orow[:, g * 384 : (g + 1) * 384], ps[:, 0:384])
                sync.dma_start(out=dst_rows[b * TT + tt], in_=orow)

        store_rows_from_T(qpT, qh_rows, b)

        # ---- K projection -> kpT [e, t], plus kh rows output -------------
        wT = load_weight_T("Wk")
        xT = load_xT(k_rows, b)
        kpT = pkt.tile([128, DC, L], FP, name="kpT", tag="kpT")
        for ec in range(DC):
            for th in range(2):
                ps = ps_ms.tile([128, 512], FP, name="ms", tag="ms")
                for dc in range(DC):
                    mm(
                        ps,
                        wT[:, dc, ec * 128 : (ec + 1) * 128],
                        xT[:, dc, th * 512 : (th + 1) * 512],
                        start=(dc == 0),
                        stop=(dc == DC - 1),
                    )
                dve.tensor_scalar_add(
                    kpT[:, ec, th * 512 : (th + 1) * 512], ps, bk_pt[:, ec : ec + 1]
                )
        store_rows_from_T(kpT, kh_rows, b)

        # ---- V projection -> vaug rows [t_in, tt, h, 65] (col 64 = ones) -
        wT = load_weight_T("Wv")
        xT = load_xT(v_rows, b)
        vaug = vpool.tile([128, TT, H, C + 1], FP, name="vaug", tag="vaug")
        dve.memset(vaug[:, :, :, C : C + 1], 1.0)
        for tt in range(TT):
            for g in range(2):
                ps = ps_ms.tile([128, 512], FP, name="ms", tag="ms")
                for dc in range(DC):
                    mm(
                        ps[:, 0:384],
                        xT[:, dc, tt * 128 : (tt + 1) * 128],
                        wT[:, dc, g * 384 : (g + 1) * 384],
                        start=(dc == 0),
                        stop=(dc == DC - 1),
                    )
                dve.tensor_add(
                    vaug[:, tt, 6 * g : 6 * (g + 1), 0:C],
                    ps[:, 0:384].rearrange("p (h c) -> p h c", c=C),
                    bvb[:, g * 384 : (g + 1) * 384].rearrange("p (h c) -> p h c", c=C),
                )
            sync.dma_start(
                out=vh_rows[b * TT + tt].rearrange("p (h c) -> p h c", c=C),
                in_=vaug[:, tt, :, 0:C],
            )

        # ---- attention per head; mixT [e, t] accumulated -----------------
        woT = load_weight_T("Wo")  # overlaps attention
        mixT = big6.tile([128, DC, L], FP, name="mixT", tag="big")
        for h in range(H):
            p0 = 64 * (h % 2)
            h2 = h // 2
            pts = []
            for kvt in range(TT):
                ps = ps_qk.tile([128, 1024], FP, name="qk", tag="qk")
                for qc in range(2):
                    mm(
                        ps[:, qc * 512 : (qc + 1) * 512],
                        kpT[p0 : p0 + 64, h2, kvt * 128 : (kvt + 1) * 128],
                        qpT[p0 : p0 + 64, h2, qc * 512 : (qc + 1) * 512],
                        start=True,
                        stop=True,
                    )
                pt = ptpool.tile([128, 1024], FP, name="pt", tag="pt")
                act.activation(pt, ps, EXP, bias=0.0, scale=float(SCALE))
                if dbg is not None and b == 0 and kvt == 0 and h in (0, 1):
                    sync.dma_start(out=dbg[f"dbg_pt{h}"].ap(), in_=pt)
                pts.append(pt)
            for qc in range(2):
                pv = ps_pv.tile([128, 512], FP, name="pv", tag="pv")
                for kvt in range(TT):
                    mm(
                        pv[0 : C + 1, :],
                        vaug[:, kvt, h, :],
                        pts[kvt][:, qc * 512 : (qc + 1) * 512],
                        start=(kvt == 0),
                        stop=(kvt == TT - 1),
                    )
                msb = mpool.tile([C + 1, 512], FP, name="msb", tag="msb")
                act.copy(msb[0:C, :], pv[0:C, :])
                den = rpool.tile([1, 512], FP, name="den", tag="rc")
                act.copy(den, pv[C : C + 1, :])  # cross-partition 64 -> 0
                rc = rpool.tile([1, 512], FP, name="rc", tag="rc")
                dve.reciprocal_approx_fast(out=rc, in_=den)
                bc = ps_ms.tile([128, 512], FP, name="ms", tag="ms")
                mm(bc[0:C, :], ones[0:1, 0:C], rc, start=True, stop=True)
                if dbg is not None and b == 0 and h == 0 and qc == 0:
                    sync.dma_start(out=dbg["dbg_msb"].ap(), in_=msb[0:C, :])
                    sync.dma_start(out=dbg["dbg_rc"].ap(), in_=rc)
                dve.tensor_mul(
                    mixT[p0 : p0 + C, h2, qc * 512 : (qc + 1) * 512],
                    msb[0:C, :],
                    bc[0:C, :],
                )

        if dbg is not None and b == 0:
            sync.dma_start(out=dbg["dbg_mixT"].ap(), in_=mixT)

        # ---- output projection -> out rows -------------------------------
        for tt in range(TT):
            orow = orows.tile([128, D], FP, name="orow", tag="orow")
            for oc in range(2):
                ps = ps_ms.tile([128, 512], FP, name="ms", tag="ms")
                for hc in range(DC):
                    mm(
                        ps[:, 0:384],
                        mixT[:, hc, tt * 128 : (tt + 1) * 128],
                        woT[:, hc, oc * 384 : (oc + 1) * 384],
                        start=(hc == 0),
                        stop=(hc == DC - 1),
                    )
                dve.tensor_add(
                    orow[:, oc * 384 : (oc + 1) * 384],
                    ps[:, 0:384],
                    bob[:, oc * 384 : (oc + 1) * 384],
                )
            sync.dma_start(out=out_rows[b * TT + tt], in_=orow)

    ctx.close()


_CACHE = {}


def _build(debug=False):
    key = ("nc", debug)
    if key in _CACHE:
        return _CACHE[key]
    nc = bacc.Bacc("TRN2", target_bir_lowering=False, debug=False)
    io = {}
    for n in ("q", "k", "v"):
        io[n] = nc.dram_tensor(n, [B, L, D], FP, kind="ExternalInput")
    for n in ("Wq", "Wk", "Wv", "Wo"):
        io[n] = nc.dram_tensor(n, [D, D], FP, kind="ExternalInput")
    for n in ("bq", "bk", "bv", "bo"):
        io[n] = nc.dram_tensor(n, [D], FP, kind="ExternalInput")
    for n in ("out", "qh", "kh", "vh"):
        io[n] = nc.dram_tensor(n, [B, L, D], FP, kind="ExternalOutput")
    dbg = None
    if debug:
        dbg = {}
        for n, shape in (
            ("dbg_pt0", [128, 1024]),
            ("dbg_pt1", [128, 1024]),
            ("dbg_msb", [C, 512]),
            ("dbg_rc", [1, 512]),
            ("dbg_mixT", [128, DC, L]),
        ):
            dbg[n] = nc.dram_tensor(n, shape, FP, kind="ExternalOutput")
    with tile.TileContext(nc) as tc:
        _emit(nc, tc, io, dbg=dbg)
    nc.compile()
    _CACHE[key] = nc
    return nc


def make_in_maps(inputs):
    a = {n: np.ascontiguousarray(np.asarray(v, dtype=np.float32)) for n, v in inputs.items()}
    in_maps = []
    for c in range(NCORES):
        m = {n: a[n] for n in ("Wq", "bq", "Wk", "bk", "Wv", "bv", "Wo", "bo")}
        for n in ("q", "k", "v"):
            m[n] = a[n][c * B : (c + 1) * B]
        in_maps.append(m)
    return in_maps


def run(inputs):
    nc = _build()
    in_maps = make_in_maps(inputs)
    res = run_bass_kernel_spmd(nc, in_maps, core_ids=list(range(NCORES)))
    full = {
        n: np.concatenate([res.results[c][n] for c in range(NCORES)], axis=0)
        for n in ("out", "qh", "kh", "vh")
    }
    N = NCORES * B
    out = full["out"]
    qh = full["qh"].reshape(N, L, H, C)
    kh = full["kh"].reshape(N, L, H, C)
    vh = full["vh"].reshape(N, L, H, C)
    return (out, qh, kh, vh), res


def kernel(**inputs):
    outs, _ = run(inputs)
    return outs


# revision 42
# speedup vs baseline: 11.0893x; 11.0893x over previous
"""Multi-head attention (projections + softmax attention + output proj) on 8
Trainium2 NeuronCores, data-parallel over the batch dim (16 batches -> 2 per
core).

Math (per batch item, H=12 heads, C=64):
    qp = q @ Wq.T + bq        (same k, v)
    S_h = (qp_h * 1/8) @ kp_h.T            [Lq, Lkv]
    P_h = softmax over kv
    mix_h = P_h @ vp_h
    out = concat_h(mix_h) @ Wo.T + bo
Outputs: (out, qh, kh, vh) where qh/kh/vh are the projected tensors reshaped
to [N, L, H, C].

Kernel layout strategy (per core):
  - activations are PE-transposed to xT [d, t]; projections produce qpT/kpT
    in [e, t] layout (heads = partition slices) and vp in row layout,
    augmented with a ones column (vaug) so the PV matmul also produces the
    softmax denominator for free.
  - S^T = K Q^T is computed directly in [kv, q] layout (no P transposes);
    exp runs on ScalarE with the 1/8 scale folded in; PV accumulates
    mix_aug^T = Vaug^T P^T in PSUM; normalization multiplies by the
    broadcast reciprocal denominator during PSUM evacuation.
  - output projection contracts mixT against WoT producing row-layout out.
"""

from contextlib import ExitStack

import numpy as np

import concourse.bass as bass
import concourse.tile as tile
from concourse import bacc, mybir
from concourse.bass_utils import run_bass_kernel_spmd
from concourse.masks import make_identity

FP = mybir.dt.float32
FR = mybir.dt.float32r  # fp32 data, PE streams at full rate (requires rounded producers)
BF = mybir.dt.bfloat16
NCORES = 8
B = 2  # batch items per core
L = 1024  # sequence length (q and kv)
D = 768  # model dim
H = 12  # heads
C = 64  # head channels
DC = D // 128  # 6 chunks of the contraction dim
TT = L // 128  # 8 token tiles per batch item
SCALE = 1.0 / np.sqrt(C).astype(np.float32)  # 0.125

EXP = mybir.ActivationFunctionType.Exp


def _rows(t):
    # [B, L, D] dram tensor -> [B*TT, 128, D] token-tile view
    return t.ap().rearrange("b l d -> (b l) d").rearrange("(t p) d -> t p d", p=128)


def _emit(nc, tc, io, dbg=None):
    ctx = ExitStack()
    sync = nc.sync
    act = nc.scalar
    dve = nc.vector
    mm = nc.tensor.matmul

    q_rows, k_rows, v_rows = _rows(io["q"]), _rows(io["k"]), _rows(io["v"])
    out_rows, qh_rows, kh_rows, vh_rows = (
        _rows(io["out"]),
        _rows(io["qh"]),
        _rows(io["kh"]),
        _rows(io["vh"]),
    )
    w_dram = {n: io[n].ap() for n in ("Wq", "Wk", "Wv", "Wo")}

    singles = ctx.enter_context(tc.tile_pool(name="singles", bufs=1))
    wpool = ctx.enter_context(tc.tile_pool(name="wpool", bufs=2))
    big6 = ctx.enter_context(tc.tile_pool(name="big6", bufs=2))
    pkt = ctx.enter_context(tc.tile_pool(name="pkt", bufs=2))
    pkt1 = ctx.enter_context(tc.tile_pool(name="pkt1", bufs=1))
    vpool = ctx.enter_context(tc.tile_pool(name="vpool", bufs=1))
    xrows = ctx.enter_context(tc.tile_pool(name="xrows", bufs=7))
    orows = ctx.enter_context(tc.tile_pool(name="orows", bufs=2))
    ptpool = ctx.enter_context(tc.tile_pool(name="ptpool", bufs=3))
    rpool = ctx.enter_context(tc.tile_pool(name="rpool", bufs=2))
    mpool = ctx.enter_context(tc.tile_pool(name="mpool", bufs=2))
    ps_qk = ctx.enter_context(tc.tile_pool(name="ps_qk", bufs=2, space="PSUM"))
    ps_pv = ctx.enter_context(tc.tile_pool(name="ps_pv", bufs=2, space="PSUM"))
    ps_ms = ctx.enter_context(tc.tile_pool(name="ps_ms", bufs=2, space="PSUM"))

    # --- constants ---------------------------------------------------------
    identity = singles.tile([128, 128], FP)
    make_identity(nc, identity)
    ones = singles.tile([1, 128], FP)
    dve.memset(ones, 1.0)
    ones96 = singles.tile([128, TT * H], FP)
    dve.memset(ones96, 1.0)
    ones_fr = singles.tile([1, 128], FR)
    dve.tensor_copy(ones_fr, ones)


    def bias_bcast(name):
        row = xrows.tile([128, D], FP, name="xrow", tag="xrow")
        sync.dma_start(out=row[0:1, :], in_=io[name].ap().rearrange("(a d) -> a d", a=1))
        bc = singles.tile([128, D], FP, name=f"{name}_bc")
        for off, w in ((0, 512), (512, 256)):
            ps = ps_ms.tile([128, 512], FP, name="ms", tag="ms")
            mm(ps[:, :w], ones[0:1, 0:128], row[0:1, off : off + w], start=True, stop=True)  # noqa: E501
            dve.tensor_copy(bc[:, off : off + w], ps[:, :w])
        return bc

    bqb = bias_bcast("bq")
    bkb = bias_bcast("bk")
    bvb = bias_bcast("bv")
    bob = bias_bcast("bo")

    uid = nc.next_id()
    wt_cache = {
        n: nc.dram_tensor(f"wtc{uid}_{n}", [128, DC, D], FR).ap()
        for n in ("Wq", "Wk", "Wv", "Wo")
    }

    def load_weight_T(wname, b, dt=None):
        dt = dt or FR
        wT = wpool.tile([128, DC, D], dt, name=f"{wname}T", tag="W")
        if b > 0:
            sync.dma_start(out=wT, in_=wt_cache[wname])
            return wT
        for g in range(2):  # groups of 3 e-chunks
            wr = []
            for j in range(3):
                t = xrows.tile([128, D], FP, name="wrow", tag="xrow")
                sync.dma_start(
                    out=t,
                    in_=w_dram[wname].rearrange("(t p) d -> t p d", p=128)[3 * g + j],
                )
                wr.append(t)
            for dc in range(DC):
                ps = ps_ms.tile([128, 512], FP, name="ms", tag="ms")
                for j in range(3):
                    nc.tensor.transpose(
                        ps[:, j * 128 : (j + 1) * 128],
                        wr[j][:, dc * 128 : (dc + 1) * 128],
                        identity,
                    )
                dve.tensor_copy(wT[:, dc, g * 384 : (g + 1) * 384], ps[:, 0:384])
        nc.gpsimd.dma_start(out=wt_cache[wname], in_=wT)
        return wT

    def load_xT(x_rows_view, b):
        """DMA activation rows, PE-transpose to xT [d_in(128), dc, t]."""
        xT = big6.tile([128, DC, L], FR, name="xT", tag="big")
        for g in range(2):  # groups of 4 token tiles
            xr = []
            for j in range(4):
                t = xrows.tile([128, D], FP, name="xrow", tag="xrow")
                sync.dma_start(out=t, in_=x_rows_view[b * TT + 4 * g + j])
                xr.append(t)
            for dc in range(DC):
                ps = ps_ms.tile([128, 512], FP, name="ms", tag="ms")
                for j in range(4):
                    nc.tensor.transpose(
                        ps[:, j * 128 : (j + 1) * 128],
                        xr[j][:, dc * 128 : (dc + 1) * 128],
                        identity,
                    )
                act.copy(xT[:, dc, g * 512 : (g + 1) * 512], ps)
        return xT

    for b in range(B):
        # ---- Q/K projections, rows orientation: fp32 rows -> DMA out,
        # ---- then PE-transpose rows -> bf16 [e, t] tiles for attention ----
        def proj_qk_rows(wT, xT, dst_rows, bias_bc, dstT, b):
            for gt in range(2):  # groups of 4 token tiles
                qr = []
                for j in range(4):
                    tt = gt * 4 + j
                    r = xrows.tile([128, D], FP, name="xrow", tag="xrow")
                    for g in range(2):  # e halves of 384
                        ps = ps_ms.tile([128, 512], FP, name="ms", tag="ms")
                        for dc in range(DC):
                            mm(
                                ps[:, 0:384],
                                xT[:, dc, tt * 128 : (tt + 1) * 128],
                                wT[:, dc, g * 384 : (g + 1) * 384],
                                start=(dc == 0),
                                stop=(dc == DC - 1),
                            )
                        dve.tensor_add(
                            r[:, g * 384 : (g + 1) * 384],
                            ps[:, 0:384],
                            bias_bc[:, g * 384 : (g + 1) * 384],
                        )
                    nc.gpsimd.dma_start(out=dst_rows[b * TT + tt], in_=r)
                    qr.append(r)
                for ec in range(DC):
                    ps = ps_ms.tile([128, 512], FP, name="ms", tag="ms")
                    for j in range(4):
                        nc.tensor.transpose(
                            ps[:, j * 128 : (j + 1) * 128],
                            qr[j][:, ec * 128 : (ec + 1) * 128],
                            identity,
                        )
                    dve.tensor_copy(dstT[:, ec, gt * 512 : (gt + 1) * 512], ps)

        wT = load_weight_T("Wq", b)
        xT = load_xT(q_rows, b)
        qpT = pkt.tile([128, DC, L], BF, name="qpT", tag="qpT")
        proj_qk_rows(wT, xT, qh_rows, bqb, qpT, b)

        wT = load_weight_T("Wk", b)
        xT = load_xT(k_rows, b)
        kpT = pkt1.tile([128, DC, L], BF, name="kpT", tag="kpT")
        proj_qk_rows(wT, xT, kh_rows, bkb, kpT, b)

        # ---- V projection -> vaug rows [t_in, tt, h, 65] (col 64 = ones) -
        wT = load_weight_T("Wv", b)
        xT = load_xT(v_rows, b)
        vaug = vpool.tile([128, TT, H, C + 1], FR, name="vaug", tag="vaug")
        dve.tensor_copy(
            vaug[:, :, :, C : C + 1],
            ones96.rearrange("p (a h c) -> p a h c", h=H, c=1),
        )
        for tt in range(TT):
            for g in range(2):
                ps = ps_ms.tile([128, 512], FP, name="ms", tag="ms")
                for dc in range(DC):
                    mm(
                        ps[:, 0:384],
                        xT[:, dc, tt * 128 : (tt + 1) * 128],
                        wT[:, dc, g * 384 : (g + 1) * 384],
                        start=(dc == 0),
                        stop=(dc == DC - 1),
                    )
                dve.tensor_add(
                    vaug[:, tt, 6 * g : 6 * (g + 1), 0:C],
                    ps[:, 0:384].rearrange("p (h c) -> p h c", c=C),
                    bvb[:, g * 384 : (g + 1) * 384].rearrange("p (h c) -> p h c", c=C),
                )
            nc.gpsimd.dma_start(
                out=vh_rows[b * TT + tt].rearrange("p (h c) -> p h c", c=C),
                in_=vaug[:, tt, :, 0:C].bitcast(FP),
            )

        # ---- attention per head; mixT [e, t] accumulated -----------------
        woT = load_weight_T("Wo", b)  # overlaps attention
        mixT = big6.tile([128, DC, L], FR, name="mixT", tag="big")
        for h in range(H):
            p0 = 64 * (h % 2)
            h2 = h // 2
            pts = []
            for kvt in range(TT):
                ps = ps_qk.tile([128, 1024], FP, name="qk", tag="qk")
                for qc in range(2):
                    mm(
                        ps[:, qc * 512 : (qc + 1) * 512],
                        kpT[p0 : p0 + 64, h2, kvt * 128 : (kvt + 1) * 128],
                        qpT[p0 : p0 + 64, h2, qc * 512 : (qc + 1) * 512],
                        start=True,
                        stop=True,
                    )
                pt = ptpool.tile([128, 1024], FR, name="pt", tag="pt")
                act.activation(pt, ps, EXP, bias=0.0, scale=float(SCALE))
                if dbg is not None and b == 0 and kvt == 0 and h in (0, 1):
                    sync.dma_start(out=dbg[f"dbg_pt{h}"].ap(), in_=pt)
                pts.append(pt)
            for qc in range(2):
                pv = ps_pv.tile([128, 512], FP, name="pv", tag="pv")
                for kvt in range(TT):
                    mm(
                        pv[0 : C + 1, :],
                        vaug[:, kvt, h, :],
                        pts[kvt][:, qc * 512 : (qc + 1) * 512],
                        start=(kvt == 0),
                        stop=(kvt == TT - 1),
                    )
                msb = mpool.tile([C + 1, 512], FP, name="msb", tag="msb")
                dve.tensor_copy(msb[0:C, :], pv[0:C, :])
                den = rpool.tile([1, 512], FP, name="den", tag="rc")
                act.copy(den, pv[C : C + 1, :])  # cross-partition 64 -> 0
                rc = rpool.tile([1, 512], FP, name="rc", tag="rc")
                dve.reciprocal_approx_fast(out=rc, in_=den)
                rcr = rpool.tile([1, 512], FR, name="rcr", tag="rc")
                dve.tensor_copy(rcr, rc)
                bc = ps_ms.tile([128, 512], FP, name="ms", tag="ms")
                mm(bc[0:C, :], ones_fr[0:1, 0:C], rcr, start=True, stop=True)
                if dbg is not None and b == 0 and h == 0 and qc == 0:
                    sync.dma_start(out=dbg["dbg_msb"].ap(), in_=msb[0:C, :])
                    sync.dma_start(out=dbg["dbg_rc"].ap(), in_=rc)
                dve.tensor_mul(
                    mixT[p0 : p0 + C, h2, qc * 512 : (qc + 1) * 512],
                    msb[0:C, :],
                    bc[0:C, :],
                )

        if dbg is not None and b == 0:
            sync.dma_start(out=dbg["dbg_mixT"].ap(), in_=mixT)

        # ---- output projection -> out rows -------------------------------
        for tt in range(TT):
            orow = orows.tile([128, D], FP, name="orow", tag="orow")
            for oc in range(2):
                ps = ps_ms.tile([128, 512], FP, name="ms", tag="ms")
                for hc in range(DC):
                    mm(
                        ps[:, 0:384],
                        mixT[:, hc, tt * 128 : (tt + 1) * 128],
                        woT[:, hc, oc * 384 : (oc + 1) * 384],
                        start=(hc == 0),
                        stop=(hc == DC - 1),
                    )
                dve.tensor_add(
                    orow[:, oc * 384 : (oc + 1) * 384],
                    ps[:, 0:384],
                    bob[:, oc * 384 : (oc + 1) * 384],
                )
            nc.gpsimd.dma_start(out=out_rows[b * TT + tt], in_=orow)

    ctx.close()


_CACHE = {}


def _build(debug=False, repeats=1):
    key = ("nc", debug, repeats)
    if key in _CACHE:
        return _CACHE[key]
    nc = bacc.Bacc("TRN2", target_bir_lowering=False, debug=False)
    io = {}
    for n in ("q", "k", "v"):
        io[n] = nc.dram_tensor(n, [B, L, D], FP, kind="ExternalInput")
    for n in ("Wq", "Wk", "Wv", "Wo"):
        io[n] = nc.dram_tensor(n, [D, D], FP, kind="ExternalInput")
    for n in ("bq", "bk", "bv", "bo"):
        io[n] = nc.dram_tensor(n, [D], FP, kind="ExternalInput")
    for n in ("out", "qh", "kh", "vh"):
        io[n] = nc.dram_tensor(n, [B, L, D], FP, kind="ExternalOutput")
    dbg = None
    if debug:
        dbg = {}
        for n, shape in (
            ("dbg_pt0", [128, 1024]),
            ("dbg_pt1", [128, 1024]),
            ("dbg_msb", [C, 512]),
            ("dbg_rc", [1, 512]),
            ("dbg_mixT", [128, DC, L]),
        ):
            dbg[n] = nc.dram_tensor(n, shape, FP, kind="ExternalOutput")
    with tile.TileContext(nc) as tc:
        for _ in range(repeats):
            _emit(nc, tc, io, dbg=dbg)
    nc.compile()
    _CACHE[key] = nc
    return nc


def make_in_maps(inputs):
    a = {n: np.ascontiguousarray(np.asarray(v, dtype=np.float32)) for n, v in inputs.items()}
    in_maps = []
    for c in range(NCORES):
        m = {n: a[n] for n in ("Wq", "bq", "Wk", "bk", "Wv", "bv", "Wo", "bo")}
        for n in ("q", "k", "v"):
            m[n] = a[n][c * B : (c + 1) * B]
        in_maps.append(m)
    return in_maps


def run(inputs):
    nc = _build()
    in_maps = make_in_maps(inputs)
    res = run_bass_kernel_spmd(nc, in_maps, core_ids=list(range(NCORES)))
    full = {
        n: np.concatenate([res.results[c][n] for c in range(NCORES)], axis=0)
        for n in ("out", "qh", "kh", "vh")
    }
    N = NCORES * B
    out = full["out"]
    qh = full["qh"].reshape(N, L, H, C)
    kh = full["kh"].reshape(N, L, H, C)
    vh = full["vh"].reshape(N, L, H, C)
    return (out, qh, kh, vh), res


def kernel(**inputs):
    outs, _ = run(inputs)
    return outs
